# revision 37
# baseline (speedup 1.0000x reference)
"""DecoderBlock on 8 NeuronCores — token-data-parallel, collective-free.

Sharding: 2 cores per batch element. Sequence tiles (128 tokens) are
interleaved: core 2b ("even") owns seq tiles {0,2,..,14} of batch b, core
2b+1 ("odd") owns {1,3,..,15}. Instead of exchanging K/V between the pair
(two AllGathers cost 15us launch + size/40GBps each and serialize the
whole core), every core computes LN1 + K/V projections for ALL 2048
tokens of its batch element locally (+~40us of PE) and runs causal
attention for its own 1024 queries over the full prefix. Zero
collectives, zero inter-core dependencies; K^T and V never leave SBUF.

Each core's x_my rows are arranged [own-parity seq tiles | other-parity
seq tiles] so the instruction stream is SPMD-uniform: queries are always
the first 1024 rows. Causal structure for local q-tile j: own-block
tiles 0..j (diagonal i==j masked tri) and other-block tiles 0..j with
the i==j tile parity-dependent:
  even core (other=odd):  excluded  -> dm1 = zeros
  odd  core (other=even): included  -> dm1 = ones
passed as input tensors (dm0 = tri for all cores).

Schedule: LN1+K/V/Q are software-pipelined in 4-tile groups (PE matmuls
hide the LayerNorm DVE/ACT work). Attention runs q-group J=0 for all 16
heads, then J=1 with the out-projection/LN2 of q-tiles 0-3 interleaved
into its (exp-bound) window; proj tiles 4-7 and the MLP follow. The
shared LayerNorm's gamma/beta are absorbed into Wq/Wk/Wv/W1 host-side.

Attention inner scheme: scoresT[k,q] on PSUM (single-shot matmuls,
causally trimmed to j >= i), one strided exp per score tile (no
max-subtraction; scores bounded ~3.3), then per-q-tile probs @ V_aug
chains (sequential per PSUM bank so accumulation groups never overlap)
with a ones column giving the softmax denominator from the same matmul.
All transposes ride the XBAR DMA-transpose path, keeping PE/PSUM free.
"""

import contextlib

import numpy as np
import ml_dtypes

B, S, D, H, HD = 4, 2048, 1024, 16, 64
R = 8                       # cores
M = (B * S) // R            # 1024 own token rows per core
M2 = 2 * M                  # 2048 rows of the full batch element
DM = D * 4                  # MLP hidden 4096
NDT = D // 128              # 8 d-tiles
NBU = D // 128              # 8 d'-bundles (2 heads each)
NMO = DM // 128             # 32 mlp-hidden tiles
NT = M // 128               # 8 own row-tiles per core
NT2 = M2 // 128             # 16 row-tiles of the batch element
EPS = 1e-5
BF = ml_dtypes.bfloat16
KVC = H * 65                # 1040 va columns (64 V + 1 ones per head)

_CACHE = {}


def _split_multi_waits(nc):
    """This walrus build allows only ONE sync-wait per instruction
    (setupSyncWait: 'Too many sync wait commands'). Move extra waits onto
    same-engine nops inserted immediately before the instruction."""
    import concourse.mybir as mybir

    for bb in nc.main_func.blocks:
        orig = list(bb.instructions)
        if not any(
            i.sync_info is not None and len(i.sync_info.on_wait) > 1
            for i in orig
        ):
            continue
        new_list = []
        for inst in orig:
            si = inst.sync_info
            if si is not None and len(si.on_wait) > 1:
                waits = list(si.on_wait)
                del si.on_wait[:]
                si.on_wait.append(waits[-1])
                for w in waits[:-1]:
                    n = nc.engines[inst.engine].nop(
                        nofuse=True, hint="wsplit"
                    )
                    cb = nc.cur_bb.bb
                    cb.instructions.remove(n.ins)
                    n.ins.sync_info = mybir.SyncInfo(on_wait=[w], on_update=[])
                    new_list.append(n.ins)
            new_list.append(inst)
        del bb.instructions[:]
        for i in new_list:
            bb.instructions.append(i)


def _build_program(mlp_act=None):
    import concourse.bass as bass
    import concourse.mybir as mybir
    import concourse.tile as tile

    f32 = mybir.dt.float32
    bf16 = mybir.dt.bfloat16
    Alu = mybir.AluOpType
    Act = mybir.ActivationFunctionType
    if mlp_act is None:
        mlp_act = Act.Gelu

    nc = bass.Bass("TRN2", target_bir_lowering=False, debug=False, num_devices=R)

    # ---- I/O -------------------------------------------------------------
    x_my = nc.dram_tensor("x_my", [M2, D], bf16, kind="ExternalInput")
    wq = nc.dram_tensor("wq", [128, NDT, D], bf16, kind="ExternalInput")
    wk = nc.dram_tensor("wk", [128, NDT, D], bf16, kind="ExternalInput")
    wv = nc.dram_tensor("wv", [128, NDT, D], bf16, kind="ExternalInput")
    wo = nc.dram_tensor("wo", [128, NDT, D], bf16, kind="ExternalInput")
    w1 = nc.dram_tensor("w1", [NMO, 128, D], bf16, kind="ExternalInput")
    w2 = nc.dram_tensor("w2", [128, NMO, D], bf16, kind="ExternalInput")
    bq_col = nc.dram_tensor("bq_col", [128, NBU], f32, kind="ExternalInput")
    bk_col = nc.dram_tensor("bk_col", [128, NBU], f32, kind="ExternalInput")
    bv_bc = nc.dram_tensor("bv_bc", [128, KVC], f32, kind="ExternalInput")
    bo_bc = nc.dram_tensor("bo_bc", [128, D], f32, kind="ExternalInput")
    b2_bc = nc.dram_tensor("b2_bc", [128, D], f32, kind="ExternalInput")
    b1_col = nc.dram_tensor("b1_col", [128, NMO], f32, kind="ExternalInput")
    dm0 = nc.dram_tensor("dm0", [128, 128], bf16, kind="ExternalInput")
    dm1 = nc.dram_tensor("dm1", [128, 128], bf16, kind="ExternalInput")
    out_my = nc.dram_tensor("out_my", [M, D], f32, kind="ExternalOutput")

    C = {}  # shared handles across phase helpers

    def ln_stats(small, xt):
        """Per-row mean/var of xt [128, D] f32 -> (rstd, nm) scalar tiles."""
        stats = small.tile([128, 2, 6], f32, tag="stats")
        nc.vector.bn_stats(out=stats[:, 0, :], in_=xt[:, 0:512])
        nc.vector.bn_stats(out=stats[:, 1, :], in_=xt[:, 512:1024])
        mv = small.tile([128, 2], f32, tag="mv")
        nc.vector.bn_aggr(out=mv, in_=stats)
        std = small.tile([128, 1], f32, tag="std")
        nc.scalar.activation(
            out=std, in_=mv[:, 1:2], func=Act.Sqrt,
            bias=C["eps_sb"][:, :], scale=1.0,
        )
        rstd = small.tile([128, 1], f32, tag="rstd")
        nc.vector.reciprocal(out=rstd, in_=std)
        nm = small.tile([128, 1], f32, tag="nm")
        nc.vector.tensor_scalar(
            out=nm, in0=mv[:, 0:1],
            scalar1=rstd, scalar2=-1.0,
            op0=Alu.mult, op1=Alu.mult,
        )
        return rstd, nm

    def phase_ln_kv_q(tc):
        """LN1 + K/V (all 2048 tokens) + Q (own 1024), software-pipelined
        in groups of 4 token-tiles so the PE matmul stream hides the
        DVE/ACT LayerNorm work. Everything stays in SBUF."""
        QT = C["QT"]
        KT, va = C["KT"], C["va"]
        hTp = C["hTp"]
        hT_g = [None] * 4    # rotating hT tiles [128, NDT, 512]
        with (
            tc.tile_pool(name="p1", bufs=4) as p1,
            tc.tile_pool(name="p1s", bufs=4) as p1s,
            tc.tile_pool(name="psP", bufs=4, space="PSUM") as psP,
        ):
            # softmax-denominator ones columns (only cols 64 of each head
            # slot); gpsimd so the DVE LayerNorm pipeline starts at once
            nc.gpsimd.memset(
                va.rearrange("p t (h c) -> p t h c", c=65)[:, :, :, 64:65],
                1.0,
            )
            wk_sb, wv_sb, wq_sb = C["wk_sb"], C["wv_sb"], C["wq_sb"]

            def ln_tile(t):
                xt = p1.tile([128, D], bf16, tag="xt")
                nc.sync.dma_start(
                    out=xt, in_=x_my[t * 128:(t + 1) * 128, :]
                )
                rstd, nm = ln_stats(p1s, xt)
                hpre = p1.tile([128, D], bf16, tag="hpre")
                nc.scalar.activation(
                    out=hpre, in_=xt, func=Act.Identity,
                    bias=nm[:, :], scale=rstd[:, :],
                )
                # XBAR DMA transpose straight into the hT layout (frees
                # PE/PSUM/DVE from the 8 transposes + 8 copies per tile)
                nc.scalar.dma_start(
                    out=hT_g[t // 4][:, :, (t % 4) * 128:(t % 4 + 1) * 128],
                    in_=hpre, transpose=True,
                )

            def k_slice(sl):  # K^T [d' bundle 128, tokens sl*512..+512]
                for bu in range(NBU):
                    ps = psP.tile([128, 512], f32, tag="psP")
                    for dt in range(NDT):
                        nc.tensor.matmul(
                            ps,
                            lhsT=wk_sb[:, dt, bu * 128:(bu + 1) * 128],
                            rhs=hT_g[sl][:, dt, :],
                            start=(dt == 0), stop=(dt == NDT - 1),
                        )
                    # PSUM->SBUF + bias on the scalar engine (DVE is the
                    # LayerNorm bottleneck in this phase)
                    nc.scalar.activation(
                        out=KT[:, bu, sl * 512:(sl + 1) * 512],
                        in_=ps, func=Act.Identity,
                        bias=C["bk_sb"][:, bu:bu + 1], scale=1.0,
                    )

            def v_tile(t):  # V [tok 128, d'] + bias into 65-slot va layout
                for sl in range(2):
                    ps = psP.tile([128, 512], f32, tag="psP")
                    for dt in range(NDT):
                        nc.tensor.matmul(
                            ps,
                            lhsT=hT_g[t // 4][:, dt,
                                              (t % 4) * 128:(t % 4 + 1) * 128],
                            rhs=wv_sb[:, dt, sl * 512:(sl + 1) * 512],
                            start=(dt == 0), stop=(dt == NDT - 1),
                        )
                    h0 = sl * 8
                    nc.vector.tensor_tensor(
                        out=va[:, t, :].rearrange(
                            "p (h c) -> p h c", c=65)[:, h0:h0 + 8, 0:64],
                        in0=ps.rearrange("p (h c) -> p h c", c=64),
                        in1=C["bvbc_sb"][:, :].rearrange(
                            "p (h c) -> p h c", c=65)[:, h0:h0 + 8, 0:64],
                        op=Alu.add,
                    )

            def q_slice(sl):  # Q^T on own rows (hT groups 0 and 1)
                for bu in range(NBU):
                    ps = psP.tile([128, 512], f32, tag="psP")
                    for dt in range(NDT):
                        nc.tensor.matmul(
                            ps,
                            lhsT=wq_sb[:, dt, bu * 128:(bu + 1) * 128],
                            rhs=hT_g[sl][:, dt, :],
                            start=(dt == 0), stop=(dt == NDT - 1),
                        )
                    nc.scalar.activation(
                        out=QT[:, bu, sl * 512:(sl + 1) * 512],
                        in_=ps, func=Act.Identity,
                        bias=C["bq_sb"][:, bu:bu + 1], scale=1.0,
                    )

            # Q slice sl spans own-token tiles 4sl..4sl+3 (= group sl)
            for g in range(4):
                hT_g[g] = hTp.tile([128, NDT, 512], bf16, tag="hTg",
                                   name=f"hTg{g}")
                for t in range(4 * g, 4 * g + 4):
                    ln_tile(t)
                k_slice(g)
                if g in (1, 2):
                    q_slice(g - 1)
                for t in range(4 * g, 4 * g + 4):
                    v_tile(t)

    def attn_head_group(h, J, psA, psO, aex, asm):
        KT, va = C["KT"], C["va"]
        QT, aosb = C["QT"], C["aosb"]
        bu, half = h // 2, h % 2
        hofs = half * 64
        dmm = (C["dm0_sb"], C["dm1_sb"])
        n_sk = 4 * J + 4
        exs = {}  # (be_, key-tile i) -> 512-col ex slice
        for be_ in range(2):
            for i0 in range(0, n_sk, 2):
                # causal trim: key tile i only matters for q-tiles j >= i.
                # Trim both jj-halves at s0 = max(0, i0-4J) so one strided
                # exp instruction covers exactly the computed columns.
                s0 = max(0, i0 - 4 * J)
                pss = psA.tile([128, 1024], mybir.dt.float32, tag="psA")
                for jj in range(2):
                    nc.tensor.matmul(
                        pss[:, jj * 512 + s0 * 128:(jj + 1) * 512],
                        lhsT=KT[hofs:hofs + 64, bu,
                                be_ * M + (i0 + jj) * 128:
                                be_ * M + (i0 + jj + 1) * 128],
                        rhs=QT[hofs:hofs + 64, bu,
                               J * 512 + s0 * 128:(J + 1) * 512],
                        start=True, stop=True,
                    )
                ex = aex.tile([128, 1024], mybir.dt.bfloat16, tag="ex",
                              name=f"ex_{h}_{J}_{be_}_{i0}")
                if s0 == 0:
                    nc.scalar.activation(out=ex, in_=pss, func=Act.Exp)
                else:
                    nc.scalar.activation(
                        out=ex.rearrange("p (j c) -> p j c",
                                         c=512)[:, :, s0 * 128:],
                        in_=pss.rearrange("p (j c) -> p j c",
                                          c=512)[:, :, s0 * 128:],
                        func=Act.Exp)
                exs[(be_, i0)] = ex[:, 0:512]
                exs[(be_, i0 + 1)] = ex[:, 512:1024]
        # per-q-tile probs@V_aug chains, one PSUM bank (zero-region) each,
        # sequential within a bank so accumulation groups never overlap
        for sq in range(4):
            j = 4 * J + sq
            pso = psO.tile([128, 65], mybir.dt.float32, tag="pso",
                           name=f"pso_{h}_{J}_{sq}")
            for be_ in range(2):
                for i in range(j + 1):
                    exsl = exs[(be_, i)][:, sq * 128:(sq + 1) * 128]
                    if i == j:
                        nc.vector.tensor_mul(
                            out=exsl, in0=exsl, in1=dmm[be_],
                        )
                    nc.tensor.matmul(
                        pso,
                        lhsT=exsl,
                        rhs=va[:, be_ * NT + i, h * 65:(h + 1) * 65],
                        start=(be_ == 0 and i == 0),
                        stop=(be_ == 1 and i == j),
                    )
            rec = asm.tile([128, 1], mybir.dt.float32, tag="rec")
            nc.vector.reciprocal(out=rec, in_=pso[:, 64:65])
            nc.vector.tensor_scalar_mul(
                out=aosb[:, j, h * 64:(h + 1) * 64],
                in0=pso[:, 0:64],
                scalar1=rec,
            )

    def make_proj_tile(p5a, p5, p5s, psB):
        """aoT transpose, out-projection + residual, LN2 -> h2T for one
        own-token tile. All elementwise work on DVE (ACT is exp-bound when
        this is interleaved with attention)."""
        aosb, h2T = C["aosb"], C["h2T"]
        wo_sb, bobc_sb = C["wo_sb"], C["bobc_sb"]

        def proj_tile(t):
            aoT_t = p5a.tile([128, NDT, 128], mybir.dt.bfloat16, tag="aoT")
            nc.scalar.dma_start(
                out=aoT_t, in_=aosb[:, t, :], transpose=True,
            )
            xt = p5.tile([128, D], mybir.dt.bfloat16, tag="xt5")
            nc.sync.dma_start(
                out=xt, in_=x_my[t * 128:(t + 1) * 128, :]
            )
            x2t = p5.tile([128, D], mybir.dt.float32, tag="x2t")
            for sl in range(2):
                psp = psB.tile([128, 512], mybir.dt.float32, tag="psB")
                for dt in range(NDT):
                    nc.tensor.matmul(
                        psp,
                        lhsT=aoT_t[:, dt, :],
                        rhs=wo_sb[:, dt, sl * 512:(sl + 1) * 512],
                        start=(dt == 0), stop=(dt == NDT - 1),
                    )
                nc.vector.tensor_tensor(
                    out=x2t[:, sl * 512:(sl + 1) * 512],
                    in0=psp,
                    in1=xt[:, sl * 512:(sl + 1) * 512],
                    op=Alu.add,
                )
            nc.vector.tensor_tensor(
                out=x2t, in0=x2t, in1=bobc_sb, op=Alu.add,
            )
            nc.sync.dma_start(
                out=C["x2_dram"][t * 128:(t + 1) * 128, :], in_=x2t
            )
            rstd, nm = ln_stats(p5s, x2t)
            h2pre = p5.tile([128, D], mybir.dt.bfloat16, tag="h2pre")
            nc.vector.tensor_scalar(
                out=h2pre, in0=x2t,
                scalar1=rstd, scalar2=nm,
                op0=Alu.mult, op1=Alu.add,
            )
            nc.scalar.dma_start(
                out=C["h2T"][t // 4][:, :, (t % 4) * 128:(t % 4 + 1) * 128],
                in_=h2pre, transpose=True,
            )

        return proj_tile

    def phase_mlp(tc):
        h2T = C["h2T"]
        with (
            tc.tile_pool(name="p5m", bufs=1) as p5m,
            tc.tile_pool(name="w1p", bufs=3) as w1p,
            tc.tile_pool(name="p5t", bufs=2) as p5t,
            tc.tile_pool(name="psB2", bufs=3, space="PSUM") as psB,
        ):
            m_sb = p5m.tile([128, NMO, M], mybir.dt.bfloat16, tag="m")
            w2_sb = p5m.tile([128, NMO, D], mybir.dt.bfloat16, tag="w2")
            # gpsimd DMA queue: don't block the w1 tile stream on nc.sync
            nc.gpsimd.dma_start(out=w2_sb, in_=w2[:, :, :])
            b2bc_sb = p5m.tile([128, D], mybir.dt.float32, tag="b2bc")
            nc.gpsimd.dma_start(out=b2bc_sb, in_=b2_bc[:, :])
            for mo in range(NMO):
                w1t = w1p.tile([128, D], mybir.dt.bfloat16, tag="w1t")
                nc.scalar.dma_start(out=w1t, in_=w1[mo, :, :])
                for sl in range(2):
                    psm = psB.tile([128, 512], mybir.dt.float32, tag="psB")
                    for dt in range(NDT):
                        nc.tensor.matmul(
                            psm,
                            lhsT=w1t[:, dt * 128:(dt + 1) * 128],
                            rhs=h2T[sl][:, dt, :],
                            start=(dt == 0), stop=(dt == NDT - 1),
                        )
                    nc.scalar.activation(
                        out=m_sb[:, mo, sl * 512:(sl + 1) * 512],
                        in_=psm, func=mlp_act,
                        bias=C["b1_sb"][:, mo:mo + 1], scale=1.0,
                    )
            for t in range(NT):
                for sl in range(2):
                    psy = psB.tile([128, 512], mybir.dt.float32, tag="psB")
                    for mo in range(NMO):
                        nc.tensor.matmul(
                            psy,
                            lhsT=m_sb[:, mo, t * 128:(t + 1) * 128],
                            rhs=w2_sb[:, mo, sl * 512:(sl + 1) * 512],
                            start=(mo == 0), stop=(mo == NMO - 1),
                        )
                    xb = p5t.tile([128, 512], mybir.dt.float32, tag="xb")
                    nc.sync.dma_start(
                        out=xb,
                        in_=C["x2_dram"][t * 128:(t + 1) * 128,
                                         sl * 512:(sl + 1) * 512],
                    )
                    ot = p5t.tile([128, 512], mybir.dt.float32, tag="ot")
                    nc.vector.tensor_tensor(
                        out=ot, in0=psy, in1=xb, op=Alu.add,
                    )
                    nc.vector.tensor_tensor(
                        out=ot, in0=ot,
                        in1=b2bc_sb[:, sl * 512:(sl + 1) * 512],
                        op=Alu.add,
                    )
                    nc.sync.dma_start(
                        out=out_my[t * 128:(t + 1) * 128,
                                   sl * 512:(sl + 1) * 512],
                        in_=ot,
                    )

    with tile.TileContext(nc) as tc:
        with contextlib.ExitStack() as es:
            dram = es.enter_context(
                tc.tile_pool(name="dram", bufs=1, space="DRAM"))
            C["x2_dram"] = dram.tile([M, D], f32, tag="x2_dram",
                                     name="x2_dram")
            consts = es.enter_context(tc.tile_pool(name="consts", bufs=1))
            for nm_, src, shp in (
                ("dm0_sb", dm0, [128, 128]),
                ("dm1_sb", dm1, [128, 128]),
            ):
                t_ = consts.tile(shp, bf16, tag=nm_, name=nm_)
                nc.gpsimd.dma_start(out=t_, in_=src[:, :])
                C[nm_] = t_
            for nm_, src, shp in (
                ("bq_sb", bq_col, [128, NBU]),
                ("bk_sb", bk_col, [128, NBU]),
                ("bvbc_sb", bv_bc, [128, KVC]),
                ("b1_sb", b1_col, [128, NMO]),
            ):
                t_ = consts.tile(shp, f32, tag=nm_, name=nm_)
                nc.gpsimd.dma_start(out=t_, in_=src[:, :])
                C[nm_] = t_
            eps_sb = consts.tile([128, 1], f32, tag="eps")
            nc.vector.memset(eps_sb, EPS)
            C["eps_sb"] = eps_sb

            # ---- LN1 + K/V/Q ------------------------------------------
            h2TP = es.enter_context(tc.tile_pool(name="h2TP", bufs=1))
            C["h2T"] = [
                h2TP.tile([128, NDT, 512], bf16, tag=f"h2T{sl}",
                          name=f"h2T{sl}")
                for sl in range(2)
            ]
            aobP = es.enter_context(tc.tile_pool(name="aob", bufs=1))
            C["aosb"] = aobP.tile([128, NT, D], bf16, tag="aosb",
                                  name="aosb")
            qtP = es.enter_context(tc.tile_pool(name="qt", bufs=1))
            C["QT"] = qtP.tile([128, NBU, M], bf16, tag="QT", name="QT")
            kvaCM = tc.tile_pool(name="kva", bufs=1)
            kvaP = kvaCM.__enter__()
            C["KT"] = kvaP.tile([128, NBU, M2], bf16, tag="KT", name="KT")
            C["va"] = kvaP.tile([128, NT2, KVC], bf16, tag="va", name="va")
            hTpCM = tc.tile_pool(name="hTp", bufs=2)
            C["hTp"] = hTpCM.__enter__()
            qkwCM = tc.tile_pool(name="qkw", bufs=1)
            qkwP = qkwCM.__enter__()
            for nm_, src in (("wk_sb", wk), ("wv_sb", wv), ("wq_sb", wq)):
                t_ = qkwP.tile([128, NDT, D], bf16, tag=nm_, name=nm_)
                nc.gpsimd.dma_start(out=t_, in_=src[:, :, :])
                C[nm_] = t_
            phase_ln_kv_q(tc)
            qkwCM.__exit__(None, None, None)
            hTpCM.__exit__(None, None, None)

            # ---- attention (J-outer) + interleaved out-proj/LN2 -------
            projCM = tc.tile_pool(name="projP", bufs=1)
            projP = projCM.__enter__()
            wo_sb = projP.tile([128, NDT, D], bf16, tag="wo", name="wo_sb")
            nc.gpsimd.dma_start(out=wo_sb, in_=wo[:, :, :])
            C["wo_sb"] = wo_sb
            bobc_sb = projP.tile([128, D], f32, tag="bo", name="bobc_sb")
            nc.gpsimd.dma_start(out=bobc_sb, in_=bo_bc[:, :])
            C["bobc_sb"] = bobc_sb
            p5aCM = tc.tile_pool(name="p5a", bufs=2)
            p5aP = p5aCM.__enter__()
            p5CM = tc.tile_pool(name="p5", bufs=2)
            p5P = p5CM.__enter__()
            p5sCM = tc.tile_pool(name="p5s", bufs=4)
            p5sP = p5sCM.__enter__()
            psBCM = tc.tile_pool(name="psB1", bufs=2, space="PSUM")
            psBP = psBCM.__enter__()
            proj_tile = make_proj_tile(p5aP, p5P, p5sP, psBP)

            aexCM = tc.tile_pool(name="aex", bufs=12)
            aexP = aexCM.__enter__()
            asmCM = tc.tile_pool(name="asm", bufs=8)
            asmP = asmCM.__enter__()
            psACM = tc.tile_pool(name="psA", bufs=2, space="PSUM")
            psAP = psACM.__enter__()
            psOCM = tc.tile_pool(name="psO", bufs=2, space="PSUM")
            psOP = psOCM.__enter__()

            for h in range(H):
                attn_head_group(h, 0, psAP, psOP, aexP, asmP)
            for h in range(H):
                attn_head_group(h, 1, psAP, psOP, aexP, asmP)
                if h % 4 == 3:
                    proj_tile(h // 4)

            psOCM.__exit__(None, None, None)
            psACM.__exit__(None, None, None)
            asmCM.__exit__(None, None, None)
            aexCM.__exit__(None, None, None)

            for t in range(4, 8):
                proj_tile(t)

            psBCM.__exit__(None, None, None)
            p5sCM.__exit__(None, None, None)
            p5CM.__exit__(None, None, None)
            p5aCM.__exit__(None, None, None)
            projCM.__exit__(None, None, None)
            kvaCM.__exit__(None, None, None)

            # ---- MLP --------------------------------------------------
            phase_mlp(tc)
    _split_multi_waits(nc)
    return nc


def _row_perm(core):
    """Global token-row indices of the OWN block of `core`, local order."""
    b, p = core // 2, core % 2
    rows = []
    for i in range(NT):
        t_seq = 2 * i + p
        base = b * S + t_seq * 128
        rows.extend(range(base, base + 128))
    return np.asarray(rows)


def _row_perm2(core):
    """Global token-row indices for x_my: [own block | other block]."""
    b, p = core // 2, core % 2
    rows = []
    for par in (p, 1 - p):
        for i in range(NT):
            t_seq = 2 * i + par
            base = b * S + t_seq * 128
            rows.extend(range(base, base + 128))
    return np.asarray(rows)


def _prep_inputs(x, Wq, Wk, Wv, bq, bk, bv, Wo, bo, W1, b1, W2, b2, gamma, beta):
    """Shard + cast host-side; returns list of per-core input dicts."""
    xf = np.ascontiguousarray(x.reshape(B * S, D)).astype(BF)
    tri = np.triu(np.ones((128, 128), np.float32)).astype(BF)
    ones = np.ones((128, 128), np.float32).astype(BF)
    zeros = np.zeros((128, 128), np.float32).astype(BF)
    bo_bc = np.ascontiguousarray(
        np.broadcast_to(bo.astype(np.float32), (128, D)))
    b2_bc = np.ascontiguousarray(
        np.broadcast_to(b2.astype(np.float32), (128, D)))

    def wt_t(w2d):  # [D, D'] -> [128, NDT, D'] (d-tile partition-major)
        dp = w2d.shape[1]
        return np.ascontiguousarray(
            np.asarray(w2d, np.float32).reshape(NDT, 128, dp)
            .transpose(1, 0, 2)).astype(BF)

    # absorb the shared LayerNorm's gamma/beta into every consumer of the
    # normalized activations: (g*h + be) @ W + b == h @ (diag(g)W) + (b + be@W)
    g = np.asarray(gamma, np.float32)
    be = np.asarray(beta, np.float32)
    Wq_all = np.concatenate([Wq[h] for h in range(H)], axis=1).astype(np.float32)
    Wk_all = np.concatenate([Wk[h] for h in range(H)], axis=1).astype(np.float32)
    Wv_all = np.concatenate([Wv[h] for h in range(H)], axis=1).astype(np.float32)
    bq_f = np.concatenate([np.asarray(bq[h], np.float32) for h in range(H)])
    bk_f = np.concatenate([np.asarray(bk[h], np.float32) for h in range(H)])
    bv_f = np.concatenate([np.asarray(bv[h], np.float32) for h in range(H)])
    bq_f = (bq_f + be @ Wq_all) * 0.125
    bk_f = bk_f + be @ Wk_all
    bv_f = bv_f + be @ Wv_all
    Wq_all = Wq_all * (0.125 * g[:, None])
    Wk_all = Wk_all * g[:, None]
    Wv_all = Wv_all * g[:, None]
    W1_f = np.asarray(W1, np.float32)
    b1_f = np.asarray(b1, np.float32) + be @ W1_f
    W1_f = W1_f * g[:, None]

    wq_t, wk_t, wv_t = wt_t(Wq_all), wt_t(Wk_all), wt_t(Wv_all)
    wo_t = wt_t(Wo)
    w1_t = np.ascontiguousarray(
        W1_f.reshape(NDT, 128, NMO, 128).transpose(2, 1, 0, 3).reshape(
            NMO, 128, D)).astype(BF)
    w2_t = np.ascontiguousarray(
        W2.reshape(NMO, 128, D).transpose(1, 0, 2)).astype(BF)

    b1_col = np.ascontiguousarray(b1_f.reshape(NMO, 128).T)
    bq_col = np.ascontiguousarray(bq_f.reshape(NBU, 128).T)
    bk_col = np.ascontiguousarray(bk_f.reshape(NBU, 128).T)
    bv_bc = np.zeros((128, KVC), np.float32)
    for h in range(H):
        bv_bc[:, h * 65:h * 65 + 64] = bv_f[h * 64:(h + 1) * 64]

    common = {
        "wq": wq_t, "wk": wk_t, "wv": wv_t,
        "wo": wo_t, "w1": w1_t, "w2": w2_t,
        "bq_col": bq_col, "bk_col": bk_col, "bv_bc": bv_bc,
        "bo_bc": bo_bc, "b2_bc": b2_bc, "b1_col": b1_col,
    }
    in_maps = []
    for r in range(R):
        p = r % 2
        in_maps.append(dict(
            common,
            x_my=np.ascontiguousarray(xf[_row_perm2(r)]),
            dm0=tri,
            dm1=(zeros if p == 0 else ones),
        ))
    return in_maps


def kernel(**inputs):
    inputs = {k: np.asarray(v) for k, v in inputs.items()}
    in_maps = _prep_inputs(**inputs)
    if "nc" not in _CACHE:
        _CACHE["nc"] = _build_program()
    from concourse.bass_utils import run_bass_kernel_spmd
    res = run_bass_kernel_spmd(_CACHE["nc"], in_maps, list(range(R)))
    _CACHE["last_res"] = res
    out = np.empty((B * S, D), np.float32)
    for r in range(R):
        out[_row_perm(r)] = res.results[r]["out_my"]
    return np.ascontiguousarray(out.reshape(B, S, D), dtype=np.float32)
